# revision 3
# baseline (speedup 1.0000x reference)
"""Trainium2 Bass kernel for nn_MoELayer_5566277616585.

MoE layer with a quirk: the gate FFN outputs H=1024 logits, top-2 indices
>= E=8 are dropped, so ~98% of tokens route nowhere.  Strategy:

Launch 1 (f32r, fast): token-parallel gate FFN on 8 cores (512 tokens each).
  Returns per-token top-8 logit slice + (max, 2nd max).  f32r has ~1e-3
  error, so launch 1 only *selects candidates* with a tau-margin superset.
Launch 2 (fp32, exact): expert-parallel FFN over candidate tokens (core c =
  expert c) PLUS an F-sharded exact fp32 recompute of the gate logits for
  all candidate tokens (1/8 of ffn dim per core).  Host combines: exact
  top-2 membership + exact weights from the recomputed logits.

SELU is composed as  selu(z) = Relu(lam*z + lam*b) + lam*alpha*min(exp(z+b),1)
  - lam*alpha, with the constant -lam*alpha folded into the *output* bias
  via  bo_adj = bo - lam*alpha*colsum(wo).
"""

import numpy as np

import jax
from jax.experimental.shard_map import shard_map
from jax.sharding import Mesh, PartitionSpec

import concourse.bass as bass
import concourse.tile as tile
from concourse import bacc, mybir
from concourse.bass2jax import _bass_exec_p, install_neuronx_cc_hook, partition_id_tensor

F32 = mybir.dt.float32
F32R = mybir.dt.float32r
AX = mybir.AxisListType
OP = mybir.AluOpType
ACT = mybir.ActivationFunctionType

B, S, H, F, E = 2, 2048, 1024, 4096, 8
N = B * S              # 4096 tokens
NCORES = 8
TOK = N // NCORES      # 512 tokens per core in launch 1
LAM = 1.0507009873554805
ALPHA = 1.6732632423543772
LAM_ALPHA = LAM * ALPHA
TAU = 0.05             # candidate margin (f32r logit max err ~2e-3)

HT = H // 128          # 8 h-tiles
FT = F // 128          # 32 f-tiles
FS = F // NCORES       # 512: gate ffn shard per core in launch 2


def _ffn_block(nc, pools, xt_tiles, wi_ap, wo_ap, lbi_t, bi_t, boa_t, out_sb,
               ntok, dtype, wi_cast, n_fshard_tiles=FT, hout=H):
    """Shared FFN body: out_sb[:ntok, :hout] = selu'(x@wi+bi) @ wo (+boa).

    xt_tiles: 8 SBUF tiles [128, ntok] (x transposed), dtype matching matmul.
    wi_ap/wo_ap: DRAM APs [H, n_fshard_tiles*128], [n_fshard_tiles*128, hout].
    lbi_t/bi_t: SBUF [128, n_fshard_tiles] per-partition biases (lam*b, b).
    boa_t: SBUF [128, hout] adjusted output bias (or None -> plain copy out).
    out_sb: SBUF tile [>=ntok, hout] result (token-major).
    dtype: matmul dtype (F32 or F32R); wi_cast: fn AP->AP for dram bitcast.
    """
    wipool, wopool, ps1, ps2, tpool, hpool = pools
    nft = n_fshard_tiles
    # --- matmul 1 + selu: h[f_tile] = selu'(x @ wi)  laid out [f, tok] ---
    hs = []
    fchunks = (nft + 3) // 4
    for fc in range(fchunks):
        f4n = min(4, nft - fc * 4)
        wts = []
        for ht in range(HT):
            w = wipool.tile([128, f4n * 128], dtype, tag="wi")
            nc.sync.dma_start(
                w[:], wi_cast(wi_ap[ht * 128:(ht + 1) * 128,
                                    fc * 512:fc * 512 + f4n * 128]))
            wts.append(w)
        for f4 in range(f4n):
            ft = fc * 4 + f4
            ps = ps1.tile([128, ntok], F32)
            for ht in range(HT):
                nc.tensor.matmul(ps[:], wts[ht][:, f4 * 128:(f4 + 1) * 128],
                                 xt_tiles[ht][:],
                                 start=(ht == 0), stop=(ht == HT - 1))
            r = tpool.tile([128, ntok], F32, tag="selu_r")
            nc.scalar.activation(r[:], ps[:], ACT.Relu,
                                 bias=lbi_t[:, ft:ft + 1], scale=LAM)
            t = tpool.tile([128, ntok], F32, tag="selu_t")
            nc.scalar.activation(t[:], ps[:], ACT.Exp,
                                 bias=bi_t[:, ft:ft + 1], scale=1.0)
            e2 = tpool.tile([128, ntok], F32, tag="selu_e")
            nc.vector.tensor_scalar(e2[:], t[:], 1.0, LAM_ALPHA,
                                    op0=OP.min, op1=OP.mult)
            h = hpool.tile([128, ntok], dtype, tag="h")
            nc.vector.tensor_add(h[:], r[:], e2[:])
            hs.append(h)
    # --- matmul 2: out[tok, hout] = h @ wo (+ boa) ---
    tchunks = (ntok + 127) // 128
    hchunks = (hout + 511) // 512
    for hc in range(hchunks):
        hn = min(512, hout - hc * 512)
        pss = [ps2.tile([min(128, ntok - tc * 128), hn], F32, tag="pss",
                        name=f"pss{hc}_{tc}") for tc in range(tchunks)]
        for ft in range(nft):
            wo_t = wopool.tile([128, hn], dtype, tag="wo")
            nc.sync.dma_start(
                wo_t[:], wi_cast(wo_ap[ft * 128:(ft + 1) * 128,
                                       hc * 512:hc * 512 + hn]))
            for tc in range(tchunks):
                tn = min(128, ntok - tc * 128)
                nc.tensor.matmul(pss[tc][:],
                                 hs[ft][:, tc * 128:tc * 128 + tn],
                                 wo_t[:],
                                 start=(ft == 0), stop=(ft == nft - 1))
        for tc in range(tchunks):
            tn = min(128, ntok - tc * 128)
            dst = out_sb[tc * 128:tc * 128 + tn, hc * 512:hc * 512 + hn]
            if boa_t is not None:
                nc.vector.tensor_add(dst, pss[tc][:],
                                     boa_t[:tn, hc * 512:hc * 512 + hn])
            else:
                nc.scalar.copy(dst, pss[tc][:])


def build_gate_program(use_f32r=True, repeat=1):
    """Launch 1: gate FFN + top-2 stats for 512 tokens/core."""
    nc = bacc.Bacc("TRN2", target_bir_lowering=False, debug=False,
                   num_devices=NCORES)
    dtype = F32R if use_f32r else F32
    cast = (lambda ap: ap.bitcast(F32R)) if use_f32r else (lambda ap: ap)

    xt = nc.dram_tensor("xt", [H, TOK], F32, kind="ExternalInput").ap()
    gwi = nc.dram_tensor("gwi", [H, F], F32, kind="ExternalInput").ap()
    gwo = nc.dram_tensor("gwo", [F, H], F32, kind="ExternalInput").ap()
    lgbi = nc.dram_tensor("lgbi", [128, FT], F32, kind="ExternalInput").ap()
    gbi = nc.dram_tensor("gbi", [128, FT], F32, kind="ExternalInput").ap()
    gboa = nc.dram_tensor("gboa", [128, H], F32, kind="ExternalInput").ap()
    l8 = nc.dram_tensor("l8", [TOK, E], F32, kind="ExternalOutput").ap()
    mm = nc.dram_tensor("mm", [TOK, 2], F32, kind="ExternalOutput").ap()

    with tile.TileContext(nc) as tc:
        import contextlib
        with contextlib.ExitStack() as ctx:
            xpool = ctx.enter_context(tc.tile_pool(name="x", bufs=HT))
            cpool = ctx.enter_context(tc.tile_pool(name="consts", bufs=1))
            wipool = ctx.enter_context(tc.tile_pool(name="wi", bufs=12))
            wopool = ctx.enter_context(tc.tile_pool(name="wo", bufs=12))
            ps1 = ctx.enter_context(tc.tile_pool(name="ps1", bufs=2, space="PSUM"))
            ps2 = ctx.enter_context(tc.tile_pool(name="ps2", bufs=4, space="PSUM"))
            tpool = ctx.enter_context(tc.tile_pool(name="tmp", bufs=3))
            hpool = ctx.enter_context(tc.tile_pool(name="h", bufs=FT))
            zpool = ctx.enter_context(tc.tile_pool(name="z", bufs=4))
            spool = ctx.enter_context(tc.tile_pool(name="small", bufs=12))
            epool = ctx.enter_context(tc.tile_pool(name="eq", bufs=2))

            def body(_i=None):
                lbi_t = cpool.tile([128, FT], F32, tag="lbi")
                nc.sync.dma_start(lbi_t[:], lgbi[:, :])
                bi_t = cpool.tile([128, FT], F32, tag="bi")
                nc.sync.dma_start(bi_t[:], gbi[:, :])
                boa_t = cpool.tile([128, H], F32, tag="boa")
                nc.sync.dma_start(boa_t[:], gboa[:, :])
                xts = []
                for ht in range(HT):
                    t = xpool.tile([128, TOK], dtype, tag="xt")
                    nc.sync.dma_start(t[:], cast(xt[ht * 128:(ht + 1) * 128, :]))
                    xts.append(t)
                zs = [zpool.tile([128, H], F32, tag="z", name=f"z{i}") for i in range(TOK // 128)]

                class _Z:
                    def __getitem__(self, idx):
                        # out_sb view: token-major [TOK, H] across 4 z tiles
                        tokslice, hslice = idx
                        tc0 = tokslice.start // 128
                        return zs[tc0][0:tokslice.stop - tokslice.start, hslice]

                _ffn_block(nc,
                           (wipool, wopool, ps1, ps2, tpool, hpool),
                           xts, gwi, gwo, lbi_t, bi_t, boa_t, _Z(),
                           TOK, dtype, cast)

                for tcb in range(TOK // 128):
                    z = zs[tcb]
                    m1 = spool.tile([128, 1], F32, tag="m1")
                    nc.vector.tensor_reduce(m1[:], z[:], AX.X, OP.max)
                    eq = epool.tile([128, H], F32, tag="eq")
                    nc.vector.tensor_scalar(eq[:], z[:], m1[:, 0:1], None,
                                            op0=OP.is_equal)
                    msk = epool.tile([128, H], F32, tag="msk")
                    nc.vector.scalar_tensor_tensor(msk[:], eq[:], -1e30, z[:],
                                                   op0=OP.mult, op1=OP.add)
                    m2 = spool.tile([128, 1], F32, tag="m2")
                    nc.vector.tensor_reduce(m2[:], msk[:], AX.X, OP.max)
                    mmt = spool.tile([128, 2], F32, tag="mmt")
                    nc.vector.tensor_copy(mmt[:, 0:1], m1[:])
                    nc.vector.tensor_copy(mmt[:, 1:2], m2[:])
                    nc.sync.dma_start(mm[tcb * 128:(tcb + 1) * 128, :], mmt[:])
                    nc.sync.dma_start(l8[tcb * 128:(tcb + 1) * 128, :],
                                      z[:, 0:E])

            if repeat > 1:
                with tc.For_i(0, repeat, 1):
                    body()
            else:
                body()

    nc.compile()
    return nc


def build_ffn_program(ecap, ucap=128, repeat=1):
    """Launch 2: expert FFN on candidates (fp32) + gate F-shard recompute."""
    nc = bacc.Bacc("TRN2", target_bir_lowering=False, debug=False,
                   num_devices=NCORES)
    ident = lambda ap: ap

    xct = nc.dram_tensor("xct", [H, ecap], F32, kind="ExternalInput").ap()
    wi = nc.dram_tensor("wi", [H, F], F32, kind="ExternalInput").ap()
    wo = nc.dram_tensor("wo", [F, H], F32, kind="ExternalInput").ap()
    lbi = nc.dram_tensor("lbi", [128, FT], F32, kind="ExternalInput").ap()
    bi = nc.dram_tensor("bi", [128, FT], F32, kind="ExternalInput").ap()
    boa = nc.dram_tensor("boa", [128, H], F32, kind="ExternalInput").ap()
    xut = nc.dram_tensor("xut", [H, ucap], F32, kind="ExternalInput").ap()
    gwis = nc.dram_tensor("gwis", [H, FS], F32, kind="ExternalInput").ap()
    gwos = nc.dram_tensor("gwos", [FS, H], F32, kind="ExternalInput").ap()
    lgbis = nc.dram_tensor("lgbis", [128, FS // 128], F32, kind="ExternalInput").ap()
    gbis = nc.dram_tensor("gbis", [128, FS // 128], F32, kind="ExternalInput").ap()
    y = nc.dram_tensor("y", [ecap, H], F32, kind="ExternalOutput").ap()
    gp = nc.dram_tensor("gp", [ucap, H], F32, kind="ExternalOutput").ap()

    with tile.TileContext(nc) as tc:
        import contextlib
        with contextlib.ExitStack() as ctx:
            xpool = ctx.enter_context(tc.tile_pool(name="x", bufs=2 * HT))
            cpool = ctx.enter_context(tc.tile_pool(name="consts", bufs=1))
            wipool = ctx.enter_context(tc.tile_pool(name="wi", bufs=16))
            wopool = ctx.enter_context(tc.tile_pool(name="wo", bufs=16))
            ps1 = ctx.enter_context(tc.tile_pool(name="ps1", bufs=2, space="PSUM"))
            ps2 = ctx.enter_context(tc.tile_pool(name="ps2", bufs=4, space="PSUM"))
            tpool = ctx.enter_context(tc.tile_pool(name="tmp", bufs=3))
            hpool = ctx.enter_context(tc.tile_pool(name="h", bufs=FT + FS // 128))
            opool = ctx.enter_context(tc.tile_pool(name="outs", bufs=2))

            def body(_i=None):
                lbi_t = cpool.tile([128, FT], F32, tag="lbi")
                nc.sync.dma_start(lbi_t[:], lbi[:, :])
                bi_t = cpool.tile([128, FT], F32, tag="bi")
                nc.sync.dma_start(bi_t[:], bi[:, :])
                boa_t = cpool.tile([128, H], F32, tag="boa")
                nc.sync.dma_start(boa_t[:], boa[:, :])
                xts = []
                for ht in range(HT):
                    t = xpool.tile([128, ecap], F32, tag="xct")
                    nc.sync.dma_start(t[:], xct[ht * 128:(ht + 1) * 128, :])
                    xts.append(t)
                y_sb = opool.tile([ecap, H], F32, tag="y")
                _ffn_block(nc,
                           (wipool, wopool, ps1, ps2, tpool, hpool),
                           xts, wi, wo, lbi_t, bi_t, boa_t, y_sb[:, :],
                           ecap, F32, ident)
                nc.sync.dma_start(y[:, :], y_sb[:, :])

                # gate F-shard recompute (no output bias -> host adds)
                lgbis_t = cpool.tile([128, FS // 128], F32, tag="lgbis")
                nc.sync.dma_start(lgbis_t[:], lgbis[:, :])
                gbis_t = cpool.tile([128, FS // 128], F32, tag="gbis")
                nc.sync.dma_start(gbis_t[:], gbis[:, :])
                xuts = []
                for ht in range(HT):
                    t = xpool.tile([128, ucap], F32, tag="xut")
                    nc.sync.dma_start(t[:], xut[ht * 128:(ht + 1) * 128, :])
                    xuts.append(t)
                gp_sb = opool.tile([ucap, H], F32, tag="gp")
                _ffn_block(nc,
                           (wipool, wopool, ps1, ps2, tpool, hpool),
                           xuts, gwis, gwos, lgbis_t, gbis_t, None, gp_sb[:, :],
                           ucap, F32, ident, n_fshard_tiles=FS // 128)
                nc.sync.dma_start(gp[:, :], gp_sb[:, :])

            if repeat > 1:
                with tc.For_i(0, repeat, 1):
                    body()
            else:
                body()

    nc.compile()
    return nc


# ---------------------------------------------------------------------------
# SPMD runner (cached jit), mirrors concourse.bass2jax.run_bass_via_pjrt
# ---------------------------------------------------------------------------

def _build_runner(nc, n_cores=NCORES):
    install_neuronx_cc_hook()
    partition_name = nc.partition_id_tensor.name if nc.partition_id_tensor else None
    in_names, out_names, out_avals, zero_shapes = [], [], [], []
    for alloc in nc.m.functions[0].allocations:
        if not isinstance(alloc, mybir.MemoryLocationSet):
            continue
        name = alloc.memorylocations[0].name
        if alloc.kind == "ExternalInput":
            if name != partition_name:
                in_names.append(name)
        elif alloc.kind == "ExternalOutput":
            out_names.append(name)
            shape = tuple(alloc.tensor_shape)
            dtype = mybir.dt.np(alloc.dtype)
            out_avals.append(jax.core.ShapedArray(shape, dtype))
            zero_shapes.append((shape, dtype))
    n_params = len(in_names)
    all_in_names = list(in_names) + list(out_names)
    if partition_name is not None:
        all_in_names.append(partition_name)
    donate = tuple(range(n_params, n_params + len(out_names)))

    def _body(*args):
        operands = list(args)
        if partition_name is not None:
            operands.append(partition_id_tensor())
        return tuple(_bass_exec_p.bind(
            *operands,
            out_avals=tuple(out_avals),
            in_names=tuple(all_in_names),
            out_names=tuple(out_names),
            lowering_input_output_aliases=(),
            sim_require_finite=True,
            sim_require_nnan=True,
            nc=nc,
        ))

    devices = jax.devices()[:n_cores]
    mesh = Mesh(np.asarray(devices), ("core",))
    sharded = jax.jit(
        shard_map(_body, mesh=mesh,
                  in_specs=(PartitionSpec("core"),) * (n_params + len(out_names)),
                  out_specs=(PartitionSpec("core"),) * len(out_names),
                  check_rep=False),
        donate_argnums=donate, keep_unused=True)

    def run(per_core_inputs):
        concat_in = [
            np.concatenate([np.ascontiguousarray(per_core_inputs[c][nm])
                            for c in range(n_cores)], axis=0)
            for nm in in_names
        ]
        concat_zeros = [np.zeros((n_cores * s[0], *s[1:]), d)
                        for s, d in zero_shapes]
        outs = sharded(*concat_in, *concat_zeros)
        return [
            {nm: np.asarray(outs[i]).reshape(n_cores, *out_avals[i].shape)[c]
             for i, nm in enumerate(out_names)}
            for c in range(n_cores)
        ]

    run.sharded = sharded
    run.in_names = in_names
    run.out_names = out_names
    return run


_CACHE = {}


def _get_gate_runner():
    if "gate" not in _CACHE:
        _CACHE["gate"] = _build_runner(build_gate_program())
    return _CACHE["gate"]


def _get_ffn_runner(ecap, ucap):
    key = ("ffn", ecap, ucap)
    if key not in _CACHE:
        _CACHE[key] = _build_runner(build_ffn_program(ecap, ucap))
    return _CACHE[key]


# ---------------------------------------------------------------------------
# Host orchestration
# ---------------------------------------------------------------------------

def _bias_tiles(b, lam_scaled):
    """[F] -> [128, F//128]; column j = b[j*128:(j+1)*128] (tile-major)."""
    nb = (LAM * b if lam_scaled else b).astype(np.float32)
    return np.ascontiguousarray(nb.reshape(-1, 128).T)


def kernel(x, gate_wi, gate_bi, gate_wo, gate_bo,
           exp_wi, exp_bi, exp_wo, exp_bo):
    x = np.asarray(x, np.float32)
    gate_wi = np.asarray(gate_wi, np.float32)
    gate_bi = np.asarray(gate_bi, np.float32)
    gate_wo = np.asarray(gate_wo, np.float32)
    gate_bo = np.asarray(gate_bo, np.float32)
    exp_wi = np.asarray(exp_wi, np.float32)
    exp_bi = np.asarray(exp_bi, np.float32)
    exp_wo = np.asarray(exp_wo, np.float32)
    exp_bo = np.asarray(exp_bo, np.float32)

    xf = x.reshape(N, H)
    xfT = np.ascontiguousarray(xf.T)  # [H, N]

    # ---- launch 1: gate + top-2 stats ----
    run1 = _get_gate_runner()
    lgbi2d = _bias_tiles(gate_bi, True)
    gbi2d = _bias_tiles(gate_bi, False)
    gbo_adj = (gate_bo.astype(np.float64)
               - LAM_ALPHA * gate_wo.astype(np.float64).sum(0)).astype(np.float32)
    gboa_rep = np.ascontiguousarray(np.broadcast_to(gbo_adj, (128, H)))
    ins1 = [{
        "xt": xfT[:, c * TOK:(c + 1) * TOK],
        "gwi": gate_wi, "gwo": gate_wo,
        "lgbi": lgbi2d, "gbi": gbi2d, "gboa": gboa_rep,
    } for c in range(NCORES)]
    res1 = run1(ins1)
    l8 = np.concatenate([r["l8"] for r in res1], 0)      # [N, 8]
    mmv = np.concatenate([r["mm"] for r in res1], 0)     # [N, 2]
    m2a = mmv[:, 1]

    # ---- candidate selection (superset, tau-margin) ----
    cand = l8 >= (m2a[:, None] - TAU)                    # [N, E]
    cand_tok = np.nonzero(cand.any(1))[0]
    nu = len(cand_tok)
    ucap = 128
    while nu > ucap:
        ucap *= 2
    expert_rows = [np.nonzero(cand[:, e])[0] for e in range(E)]
    maxc = max((len(r) for r in expert_rows), default=1)
    ecap = 32
    while maxc > ecap:
        ecap *= 2

    # ---- launch 2: expert FFNs + exact gate recompute ----
    run2 = _get_ffn_runner(ecap, ucap)
    xut = np.zeros((H, ucap), np.float32)
    xut[:, :nu] = xfT[:, cand_tok]
    ins2 = []
    for c in range(NCORES):
        rows = expert_rows[c]
        xct = np.zeros((H, ecap), np.float32)
        xct[:, :len(rows)] = xfT[:, rows]
        boa_c = (exp_bo[c].astype(np.float64)
                 - LAM_ALPHA * exp_wo[c].astype(np.float64).sum(0)).astype(np.float32)
        sl = slice(c * FS, (c + 1) * FS)
        ins2.append({
            "xct": xct,
            "wi": exp_wi[c], "wo": exp_wo[c],
            "lbi": _bias_tiles(exp_bi[c], True),
            "bi": _bias_tiles(exp_bi[c], False),
            "boa": np.ascontiguousarray(np.broadcast_to(boa_c, (128, H))),
            "xut": xut,
            "gwis": np.ascontiguousarray(gate_wi[:, sl]),
            "gwos": np.ascontiguousarray(gate_wo[sl, :]),
            "lgbis": _bias_tiles(LAM * gate_bi[sl], False),
            "gbis": _bias_tiles(gate_bi[sl], False),
        })
    res2 = run2(ins2)

    # ---- exact logits for candidate tokens, exact top-2 + weights ----
    gsum = np.zeros((ucap, H), np.float64)
    for c in range(NCORES):
        gsum += res2[c]["gp"].astype(np.float64)
    logits_u = gsum[:nu] + (gate_bo.astype(np.float64)
                            - LAM_ALPHA * gate_wo.astype(np.float64).sum(0))
    part = np.partition(logits_u, (H - 2, H - 1), axis=1)
    m1x, m2x = part[:, -1], part[:, -2]
    denom = m1x + m2x
    upos = np.full(N, -1, np.int64)
    upos[cand_tok] = np.arange(nu)

    out = np.zeros((N, H), np.float64)
    for e in range(E):
        rows = expert_rows[e]
        if len(rows) == 0:
            continue
        ye = res2[e]["y"][:len(rows)].astype(np.float64)
        pu = upos[rows]
        le = logits_u[pu, e]
        routed = le >= m2x[pu]
        wgt = np.where(routed, le / denom[pu], 0.0)
        out[rows] += wgt[:, None] * ye
    return out.reshape(B, S, H).astype(np.float32)
